# revision 27
# baseline (speedup 1.0000x reference)
"""Multi-head attention (RoPE-by-head variant) on 8 TRN2 NeuronCores.

Sharding: tensor-parallel over heads. Core c owns heads [4c, 4c+4):
  - computes q/k/v projections for its 512 features (transposed layouts),
  - causal attention for its 4 heads entirely in SBUF,
  - AllGather of per-core attention outputs (bf16, [512, S] each -> [4096, S]),
  - output projection for its 512 OUTPUT columns (no reduce needed),
  - host concatenates column slices and adds bo.

RoPE here is indexed by HEAD (not position) in the reference, so it is a
fixed per-head 2x2 rotation of feature pairs == a linear map folded into
Wq/Wk (and bq/bk) on the host, exactly. The 1/sqrt(HD) score scale is
folded into Wq as well.

Softmax skips max-subtraction: scores are ~N(0, 1.64) so |score| < 40 with
overwhelming margin; exp() in fp32 is safe and matches softmax exactly in
exact arithmetic. Masked entries get -1e30 -> exp == 0.
"""

import math
from contextlib import ExitStack

import ml_dtypes
import numpy as np

import concourse.bass as bass
import concourse.mybir as mybir
import concourse.tile as tile
from concourse import bacc, bass_utils
from concourse.masks import make_identity
from concourse.tile_rust import add_dep_helper

# Problem dims (hardcoded per contract).
B, S, D, H, HD = 1, 2048, 4096, 32, 128
NCORES = 8
HPC = H // NCORES          # heads per core = 4
FPC = HPC * HD             # features per core = 512
ROPE_BASE = 10000.0
BLK = 512                  # key-block width for QK matmuls
P = 128                    # partitions

BF16 = mybir.dt.bfloat16
FP32 = mybir.dt.float32


# ---------------------------------------------------------------- builder --

def build_nc(s=S, hpc=HPC, ncores=NCORES, compute_dt=BF16):
    """Build the SPMD Bass program (identical on all cores; data differs)."""
    fpc = hpc * HD
    d = ncores * fpc                 # model dim (square weights)
    kc_n = d // P                    # contraction chunks for projections
    nq = s // P                      # 128-row query tiles
    sh_w = s // 2                    # sequence half width (projection passes)
    sbw = min(BLK, sh_w)             # s-block width inside a half
    nsb = sh_w // sbw
    n_fc = fpc // P                  # feature chunks per core (4)

    nc = bacc.Bacc(
        "TRN2", target_bir_lowering=False, debug=False, num_devices=ncores
    )

    # Inputs (bf16 unless noted)
    qT = nc.dram_tensor("qT", [d, s], compute_dt, kind="ExternalInput")
    kT = nc.dram_tensor("kT", [d, s], compute_dt, kind="ExternalInput")
    vT = nc.dram_tensor("vT", [d, s], compute_dt, kind="ExternalInput")
    wqT = nc.dram_tensor("wqT", [d, fpc], compute_dt, kind="ExternalInput")
    wkT = nc.dram_tensor("wkT", [d, fpc], compute_dt, kind="ExternalInput")
    wvT = nc.dram_tensor("wvT", [d, fpc], compute_dt, kind="ExternalInput")
    woT = nc.dram_tensor("woT", [d, fpc], compute_dt, kind="ExternalInput")
    # per-partition biases for q/k/v proj, one column per (proj, f-chunk)
    bqkv = nc.dram_tensor("bqkv", [P, 3 * n_fc], FP32, kind="ExternalInput")
    # transposed causal masks: [diag | full+diag] = [128, 3*128]
    masks = nc.dram_tensor("masks", [P, 3 * P], FP32, kind="ExternalInput")
    # Output: transposed slice yT = (out columns [c*fpc,(c+1)*fpc)).T
    yT = nc.dram_tensor("yT", [fpc, s], FP32, kind="ExternalOutput")

    with tile.TileContext(nc) as tc, ExitStack() as ctx:
        const = ctx.enter_context(tc.tile_pool(name="const", bufs=1))
        persist = ctx.enter_context(tc.tile_pool(name="persist", bufs=1))

        ident = const.tile([P, P], compute_dt)
        make_identity(nc, ident)
        mask_sb = const.tile([P, 3 * P], FP32)
        nc.sync.dma_start(out=mask_sb, in_=masks[:, :])
        bias_sb = const.tile([P, 3 * n_fc], FP32)
        nc.sync.dma_start(out=bias_sb, in_=bqkv[:, :])

        # Persistent SBUF tensors
        qpT = [persist.tile([P, s], compute_dt, name=f"qpT{f}") for f in range(n_fc)]
        kpT = [persist.tile([P, s], compute_dt, name=f"kpT{f}") for f in range(n_fc)]
        attnT = [persist.tile([P, s], compute_dt, name=f"attnT{f}") for f in range(hpc)]
        # attention output, natural layout [sq, HD] blocks per query tile
        attnN = [persist.tile([P, nq * HD], compute_dt, name=f"attnN{h}")
                 for h in range(hpc)]
        # vp: natural layout per head, HD+1 cols per s-block (last col = 1.0
        # so PV's matmul also produces the softmax denominator)
        vp = [persist.tile([P, nq * (HD + 1)], compute_dt, name=f"vp{h}")
              for h in range(hpc)]
        for h in range(hpc):
            ones_col = vp[h].rearrange("p (t c) -> p t c", c=HD + 1)[:, :, HD:HD + 1]
            nc.vector.memset(ones_col, 1.0)

        # ---------------- Phase 1: q/k/v projections (transposed outputs) --
        projs = [(qT, wqT, qpT, 0), (kT, wkT, kpT, 1), (vT, wvT, None, 2)]
        vpT = []  # temporary transposed v-projection
        with tc.tile_pool(name="vpT_pool", bufs=1) as vpT_pool:
          vpT = [vpT_pool.tile([P, s], compute_dt, name=f"vpT{f}")
                 for f in range(n_fc)]
          with tc.tile_pool(name="xw", bufs=3) as xw, \
               tc.tile_pool(name="ps_proj", bufs=1, space="PSUM") as ps_proj:
            for x_dram, w_dram, outs, pidx in projs:
                if outs is None:
                    outs = vpT
                for sh in range(2):
                    ps = [[ps_proj.tile([P, sbw], FP32, name=f"pp{f}_{b}",
                                        tag=f"pp{f}_{b}")
                           for b in range(nsb)] for f in range(n_fc)]
                    for kc in range(kc_n):
                        x_t = xw.tile([P, sh_w], compute_dt, name="x_t",
                                      tag=f"x{pidx}")
                        nc.sync.dma_start(
                            out=x_t,
                            in_=x_dram[kc * P:(kc + 1) * P,
                                       sh * sh_w:(sh + 1) * sh_w])
                        w_t = xw.tile([P, fpc], compute_dt, name="w_t",
                                      tag=f"w{pidx}")
                        nc.sync.dma_start(
                            out=w_t, in_=w_dram[kc * P:(kc + 1) * P, :])
                        for f in range(n_fc):
                            for b in range(nsb):
                                nc.tensor.matmul(
                                    ps[f][b],
                                    lhsT=w_t[:, f * P:(f + 1) * P],
                                    rhs=x_t[:, b * sbw:(b + 1) * sbw],
                                    start=(kc == 0), stop=(kc == kc_n - 1))
                    for f in range(n_fc):
                        for b in range(nsb):
                            col = sh * sh_w + b * sbw
                            nc.scalar.activation(
                                outs[f][:, col:col + sbw], ps[f][b],
                                mybir.ActivationFunctionType.Identity,
                                bias=bias_sb[:, pidx * n_fc + f:
                                             pidx * n_fc + f + 1])

          # ------------- Phase 2: transpose vpT -> vp (natural layout) --
          with tc.tile_pool(name="ps_tr0", bufs=2, space="PSUM") as ps_tr0:
              for h in range(hpc):
                  for st in range(nq):
                      tr = ps_tr0.tile([P, P], compute_dt, name="tr0", tag="tr0")
                      nc.tensor.transpose(
                          tr, vpT[h][:, st * P:(st + 1) * P], ident)
                      nc.vector.tensor_copy(
                          vp[h][:, st * (HD + 1):st * (HD + 1) + HD], tr)

        # ---------------- Phase 3: causal attention, per (query tile, head) --
        # AllGather chunk widths: 512s then 256s, so the last (serial-tail)
        # chunks are small; boundaries align with query-tile pairs
        chunk_ws, rem = [], s
        while rem > 0:
            w = 512 if rem >= 1024 else max(256, 2 * P)
            w = min(w, rem)
            chunk_ws.append(w)
            rem -= w
        n_ag = len(chunk_ws)
        cum = []
        acc = 0
        for w in chunk_ws:
            acc += w
            cum.append(acc)
        dram_ctx = ExitStack()
        dram_pool = dram_ctx.enter_context(
            tc.tile_pool(name="dram", bufs=1, space="DRAM"))
        ag_in = [dram_pool.tile([fpc, chunk_ws[x]], compute_dt,
                                name=f"ag_in{x}") for x in range(n_ag)]
        ag_out = [dram_pool.tile([ncores * fpc, chunk_ws[x]], compute_dt,
                                 name=f"ag_out{x}", addr_space="Shared")
                  for x in range(n_ag)]

        # out-proj weights: prefetch during attention (DMA is idle there)
        wo_sb = [persist.tile([P, fpc], compute_dt, name=f"wo{kc}")
                 for kc in range(kc_n)]
        for kc in range(kc_n):
            nc.sync.dma_start(out=wo_sb[kc], in_=woT[kc * P:(kc + 1) * P, :])

        # Scores are computed pre-transposed: scT[sk, sq] = kpT_t^T-style
        # matmul, so exp output IS the PV stationary operand (no transposes),
        # PV output is in natural [sq, hd] layout, the ones-column of vp
        # yields the softmax denominator, and normalization folds into the
        # PSUM->SBUF copy as a per-partition activation scale.
        # Query tiles are processed in PAIRS (N=256 moving) for PE efficiency.
        with tc.tile_pool(name="ps_sc", bufs=3, space="PSUM") as ps_sc, \
             tc.tile_pool(name="ps_pv", bufs=1, space="PSUM") as ps_pv, \
             tc.tile_pool(name="ps_tr", bufs=2, space="PSUM") as ps_tr, \
             tc.tile_pool(name="ps_y", bufs=1, space="PSUM") as ps_y, \
             tc.tile_pool(name="ag_sb_pool", bufs=1) as ag_sb_pool, \
             tc.tile_pool(name="ysb_pool", bufs=3) as ysb_pool, \
             tc.tile_pool(name="probs_pool", bufs=4) as probs_pool, \
             tc.tile_pool(name="small", bufs=4) as small:
            def emit_outproj(cq, marker):
                # out-projection of one AG chunk:
                # yT[j, s] = sum_f Wo_c[j, f] * ag_out[cq][f, s]
                # `marker`: attention instruction this chunk's work must
                # trail in the engine streams (the scheduler's collective
                # cost model is optimistic; without this it slots these
                # matmuls mid-attention and the PE FIFO stalls on the
                # AllGather)
                w = chunk_ws[cq]
                col0 = cum[cq] - w
                ybw = min(BLK, w)
                ag_sb = [ag_sb_pool.tile([P, w], compute_dt,
                                         name=f"ag{kc}", tag=f"ag{kc}")
                         for kc in range(kc_n)]
                for kc in range(kc_n):
                    dma = nc.sync.dma_start(
                        out=ag_sb[kc],
                        in_=ag_out[cq][kc * P:(kc + 1) * P, :])
                    if kc == 0 and marker is not None:
                        add_dep_helper(dma.ins, marker,
                                       reason="agload trails attn")
                for jm in range(n_fc):
                    for b in range(w // ybw):
                        psy = ps_y.tile([P, ybw], FP32, name="psy", tag="psy")
                        for kc in range(kc_n):
                            mm = nc.tensor.matmul(
                                psy,
                                lhsT=wo_sb[kc][:, jm * P:(jm + 1) * P],
                                rhs=ag_sb[kc][:, b * ybw:(b + 1) * ybw],
                                start=(kc == 0), stop=(kc == kc_n - 1))
                            if kc == 0 and marker is not None:
                                add_dep_helper(mm.ins, marker,
                                               reason="outproj trails attn")
                        ysb = ysb_pool.tile([P, ybw], FP32, name="ysb",
                                            tag="ysb")
                        nc.vector.tensor_copy(ysb, psy)
                        nc.sync.dma_start(
                            out=yT[jm * P:(jm + 1) * P,
                                   col0 + b * ybw:col0 + (b + 1) * ybw],
                            in_=ysb)

            chunk_marker = [None] * n_ag
            for jp in range(nq // 2):
                i0, i1 = 2 * jp, 2 * jp + 1
                for h in range(hpc):
                    pv0 = ps_pv.tile([P, HD + 1], FP32, name="pv0", tag="pv0")
                    pv1 = ps_pv.tile([P, HD + 1], FP32, name="pv1", tag="pv1")
                    for t in range(i1 + 1):
                        scT = ps_sc.tile([P, 2 * P], FP32, name="scT",
                                         tag="scT")
                        nc.tensor.matmul(
                            scT,
                            lhsT=kpT[h][:, t * P:(t + 1) * P],
                            rhs=qpT[h][:, i0 * P:(i0 + 2) * P],
                            start=True, stop=True)
                        if t == i0:  # diagonal of the i0 half
                            nc.vector.tensor_add(
                                scT[:, 0:P], scT[:, 0:P], mask_sb[:, 0:P])
                        if t == i1:  # i0 half fully masked, i1 half diagonal
                            nc.vector.tensor_add(
                                scT, scT, mask_sb[:, P:3 * P])
                        pT = probs_pool.tile([P, 2 * P], compute_dt,
                                             name="pT", tag="pT")
                        nc.scalar.activation(
                            pT, scT, mybir.ActivationFunctionType.Exp)
                        vslice = vp[h][:, t * (HD + 1):(t + 1) * (HD + 1)]
                        nc.tensor.matmul(
                            pv0, lhsT=pT[:, 0:P], rhs=vslice,
                            start=(t == 0), stop=(t == i1))
                        nc.tensor.matmul(
                            pv1, lhsT=pT[:, P:2 * P], rhs=vslice,
                            start=(t == 0), stop=(t == i1))
                    for iq, pvx in ((i0, pv0), (i1, pv1)):
                        recip = small.tile([P, 1], FP32, name="recip",
                                           tag="recip")
                        nc.vector.reciprocal(recip, pvx[:, HD:HD + 1])
                        last_copy = nc.scalar.activation(
                            attnN[h][:, iq * HD:(iq + 1) * HD],
                            pvx[:, 0:HD],
                            mybir.ActivationFunctionType.Identity,
                            scale=recip)
                # at each chunk boundary: transpose the finished columns of
                # attnN into attnT, ship them, AllGather (overlapped)
                if (i1 + 1) * P in cum:
                    cq = cum.index((i1 + 1) * P)
                    w = chunk_ws[cq]
                    col0 = cum[cq] - w
                    for h in range(hpc):
                        for st in range(col0 // P, cum[cq] // P):
                            tr = ps_tr.tile([P, P], compute_dt, name="tr",
                                            tag="tr")
                            nc.tensor.transpose(
                                tr, attnN[h][:, st * HD:(st + 1) * HD], ident)
                            nc.vector.tensor_copy(
                                attnT[h][:, st * P:(st + 1) * P], tr)
                        nc.sync.dma_start(
                            out=ag_in[cq][h * P:(h + 1) * P, :],
                            in_=attnT[h][:, col0:cum[cq]])
                    nc.gpsimd.collective_compute(
                        "AllGather", mybir.AluOpType.bypass,
                        replica_groups=[list(range(ncores))],
                        ins=[ag_in[cq][:, :]], outs=[ag_out[cq][:, :]])
                    chunk_marker[cq] = last_copy.ins

            # out-projection: chunk cq's matmuls trail the END of attention
            # chunk cq+1, by which time its AllGather has completed
            for cq in range(n_ag):
                emit_outproj(cq, chunk_marker[min(cq + 1, n_ag - 1)]
                             if cq < n_ag - 1 else None)
        dram_ctx.close()
    nc.compile()
    return nc


# ------------------------------------------------------------- host side --

def _rope_fold(W, bvec, n_heads, scale):
    """Fold head-indexed RoPE rotation (and scale) into projection weights."""
    inv = 1.0 / (ROPE_BASE ** (np.arange(0, HD, 2, dtype=np.float32) / HD))
    ang = np.arange(n_heads, dtype=np.float32)[:, None] * inv[None, :]
    cos = np.cos(ang)[:, :, None]   # [H, HD/2, 1]
    sin = np.sin(ang)[:, :, None]
    Wr = W.reshape(n_heads, HD // 2, 2, -1).astype(np.float32)
    w0, w1 = Wr[:, :, 0, :], Wr[:, :, 1, :]
    out = np.empty_like(Wr)
    out[:, :, 0, :] = cos * w0 - sin * w1
    out[:, :, 1, :] = sin * w0 + cos * w1
    Wf = out.reshape(W.shape) * scale
    br = bvec.reshape(n_heads, HD // 2, 2).astype(np.float32)
    cos2, sin2 = cos[:, :, 0], sin[:, :, 0]
    bout = np.empty_like(br)
    bout[:, :, 0] = cos2 * br[:, :, 0] - sin2 * br[:, :, 1]
    bout[:, :, 1] = sin2 * br[:, :, 0] + cos2 * br[:, :, 1]
    bf = bout.reshape(bvec.shape) * scale
    return Wf, bf


def _make_masks():
    # transposed-score masks: scT[sk_r, sq_c] valid iff sk <= sq i.e. r <= c
    r = np.arange(P, dtype=np.int64)[:, None]
    c = np.arange(P, dtype=np.int64)[None, :]
    diag = np.where(r <= c, 0.0, -1e30).astype(np.float32)
    full = np.full((P, P), -1e30, np.float32)
    return np.concatenate([diag, full, diag], axis=1)  # [128, 384]


def _bf16(x):
    return np.ascontiguousarray(np.asarray(x, dtype=np.float32)).astype(
        ml_dtypes.bfloat16)


_NC_CACHE = {}


def _get_nc():
    if "nc" not in _NC_CACHE:
        _NC_CACHE["nc"] = build_nc()
    return _NC_CACHE["nc"]


def prepare_in_maps(q, k, v, Wq, bq, Wk, bk, Wv, bv, Wo, bo):
    q = np.asarray(q, np.float32)
    k = np.asarray(k, np.float32)
    v = np.asarray(v, np.float32)
    Wq = np.asarray(Wq, np.float32)
    Wk = np.asarray(Wk, np.float32)
    Wv = np.asarray(Wv, np.float32)
    Wo = np.asarray(Wo, np.float32)
    bq = np.asarray(bq, np.float32)
    bk = np.asarray(bk, np.float32)
    bv = np.asarray(bv, np.float32)

    scale = 1.0 / math.sqrt(HD)
    Wqf, bqf = _rope_fold(Wq, bq, H, scale)
    Wkf, bkf = _rope_fold(Wk, bk, H, 1.0)

    qT = _bf16(q[0].T)
    kT = _bf16(k[0].T)
    vT = _bf16(v[0].T)
    masks = _make_masks()

    in_maps = []
    for c in range(NCORES):
        sl = slice(c * FPC, (c + 1) * FPC)
        bias = np.stack(
            [bqf[sl].reshape(4, P)[f] for f in range(4)]
            + [bkf[sl].reshape(4, P)[f] for f in range(4)]
            + [bv[sl].reshape(4, P)[f] for f in range(4)], axis=1
        ).astype(np.float32)  # [128, 12]
        in_maps.append({
            "qT": qT, "kT": kT, "vT": vT,
            "wqT": _bf16(Wqf[sl].T), "wkT": _bf16(Wkf[sl].T),
            "wvT": _bf16(Wv[sl].T), "woT": _bf16(Wo[sl].T),
            "bqkv": np.ascontiguousarray(bias), "masks": masks,
        })
    return in_maps


def postprocess(results, bo):
    bo = np.asarray(bo, np.float32)
    out = np.concatenate(
        [np.asarray(results[c]["yT"], np.float32).T
         for c in range(NCORES)], axis=1)
    out = out + bo[None, :]
    return out[None].astype(np.float32)


def kernel(q, k, v, Wq, bq, Wk, bk, Wv, bv, Wo, bo):
    in_maps = prepare_in_maps(q, k, v, Wq, bq, Wk, bk, Wv, bv, Wo, bo)
    nc = _get_nc()
    res = bass_utils.run_bass_kernel_spmd(
        nc, in_maps, core_ids=list(range(NCORES)))
    return postprocess(res.results, bo)
